# revision 9
# baseline (speedup 1.0000x reference)
"""AffineTransformLayer (nearest-neighbor warp + mask + max over transforms)
as a Trainium2 Bass kernel, SPMD over 8 NeuronCores.

Sharding: core k handles batch b = k//2, output row-half k%2 (128 rows).
No collectives; each core's output slice is disjoint.

Host side computes, per (b, n, h, w), the flat source-pixel index and a fused
weight (resized mask where the source coordinate is in-bounds, else 0) using
float32 op-for-op the same arithmetic as the reference (round-half-even via
np.round == jnp.round; per-op IEEE rounding matches eager jax on any backend).
The device then does the heavy part: 327,680 256-byte pixel gathers per core
via gpsimd dma_gather straight from HBM, a broadcast multiply by the weight,
and a running elementwise max over the N transforms.

dma_gather constraints handled here (hardware-verified):
 - idx dtype int16: gather base is placed mid-image so idx = pixel - 32768
   covers all 65536 pixels in signed range (ucode multiplies signed).
 - trailing negative idxs are dropped by the ucode: every 1024-slot chunk
   ends with a sentinel idx 0 (weight 0), so the last idx is never negative.
 - per-instruction descriptor-ring cap: 1024 idxs per dma_gather.
"""

import sys

sys.path.insert(0, "/opt/trn_rl_repo")

import numpy as np

B, H, W, C = 4, 256, 256, 64
N = 10
HM, WM = 64, 64
NPIX = H * W          # pixels per image
HALF = NPIX // 2      # pixels per core's output half / gather mid-base
ROWS = H // 2         # output rows per core

CHUNK = 1024          # idx slots per dma_gather instruction
REAL = CHUNK - 1      # real pixels per chunk (last slot = sentinel)
NCH = -(-HALF // REAL)  # chunks per transform (33)
NI = NCH * N          # gather instructions per core (330)
GCOLS = CHUNK // 128  # gather tile free cols (8)

_compiled = None


def _host_index_weights(transforms, mask):
    """Exact replication of the reference's coordinate math in float32.

    Returns flat source index [B,N,H,W] int32 and fused weight [B,N,H,W] f32.
    """
    one = np.float32(1.0)
    # affine_mul is all ones for H=W=256, INIT_SIZE=(256,256); keep the divide
    # so the float ops mirror the reference exactly.
    p = (transforms.astype(np.float32) / one).astype(np.float32)
    X = np.arange(W, dtype=np.float32)[None, None, None, :]
    Y = np.arange(H, dtype=np.float32)[None, None, :, None]

    def coef(i):
        return p[:, :, i][:, :, None, None]

    k = coef(6) * X + coef(7) * Y + one
    in_x = ((coef(0) * X + coef(1) * Y) + coef(2)) / k
    in_y = ((coef(3) * X + coef(4) * Y) + coef(5)) / k
    ix = np.round(in_x).astype(np.int32)
    iy = np.round(in_y).astype(np.int32)
    valid = (ix >= 0) & (ix < W) & (iy >= 0) & (iy < H)
    ixc = np.clip(ix, 0, W - 1)
    iyc = np.clip(iy, 0, H - 1)
    flat = iyc * W + ixc  # [B,N,H,W]

    # nearest-neighbor mask resize [B,N,64,64] -> [B,N,256,256]
    ys = np.minimum(np.arange(H) * HM // H, HM - 1)
    xs = np.minimum(np.arange(W) * WM // W, WM - 1)
    m_up = mask[:, :, ys][:, :, :, xs]  # [B,N,H,W]
    wgt = np.where(valid, m_up, np.float32(0.0)).astype(np.float32)
    return flat, wgt


def _build_program():
    import concourse.bass as bass
    import concourse.tile as tile
    from concourse import bacc, mybir

    nc = bacc.Bacc("TRN2", target_bir_lowering=False, debug=False, num_swdge_queues=4)
    # split the 16MB image input in two: walrus io-DGE pages inputs at 4KB per
    # descriptor and its 16-bit semaphore wait overflows beyond ~16MB/tensor.
    xt = nc.dram_tensor("xt", [HALF, C], mybir.dt.float32, kind="ExternalInput").ap()
    xbo = nc.dram_tensor("xbo", [HALF, C], mybir.dt.float32, kind="ExternalInput").ap()
    xb = nc.dram_tensor("xfull", [NPIX, C], mybir.dt.float32).ap()
    idxw = nc.dram_tensor(
        "idxw", [NCH, 128, N * (CHUNK // 16)], mybir.dt.int16, kind="ExternalInput"
    ).ap()
    wgtd = nc.dram_tensor(
        "wgtd", [NCH, 128, N * GCOLS], mybir.dt.float32, kind="ExternalInput"
    ).ap()
    outd = nc.dram_tensor(
        "outd", [NCH, 128, GCOLS, C], mybir.dt.float32, kind="ExternalOutput"
    ).ap()

    IDXF = CHUNK // 16
    with tile.TileContext(nc) as tc:
        with (
            tc.tile_pool(name="idxp", bufs=3) as idxp,
            tc.tile_pool(name="wp", bufs=3) as wp,
            tc.tile_pool(name="gp", bufs=6) as gp,
            tc.tile_pool(name="tp", bufs=3) as tp,
            tc.tile_pool(name="accp", bufs=3) as accp,
        ):
            nc.sync.dma_start(xb[:HALF, :], xt[:, :])
            nc.sync.dma_start(xb[HALF:, :], xbo[:, :])
            for c in range(NCH):
                it = idxp.tile([128, N * IDXF], mybir.dt.int16, tag="idx")
                nc.sync.dma_start(it[:], idxw[c, :, :])
                wt = wp.tile([128, N * GCOLS], mybir.dt.float32, tag="w")
                nc.sync.dma_start(wt[:], wgtd[c, :, :])
                acc = accp.tile([128, GCOLS, C], mybir.dt.float32, tag="acc")
                for n in range(N):
                    i = c * N + n
                    g = gp.tile([128, GCOLS, C], mybir.dt.float32, tag="g")
                    nc.gpsimd.dma_gather(
                        g[:, :, :],
                        xb[HALF:, :],
                        it[:, n * IDXF : (n + 1) * IDXF],
                        num_idxs=CHUNK,
                        num_idxs_reg=CHUNK,
                        elem_size=C,
                        queue_num=i % 4,
                    )
                    # weight multiply: per-column per-partition scalar (walrus
                    # rejects stride-0 broadcast APs on tensor_tensor). Split
                    # columns across DVE and ACT so neither engine binds.
                    if n == 0:
                        tgt = acc
                    else:
                        tgt = tp.tile([128, GCOLS, C], mybir.dt.float32, tag="t")
                    for col in range(GCOLS):
                        w1 = wt[:, n * GCOLS + col : n * GCOLS + col + 1]
                        if col % 2 == 0:
                            nc.vector.tensor_scalar(
                                tgt[:, col, :],
                                g[:, col, :],
                                w1,
                                None,
                                mybir.AluOpType.mult,
                            )
                        else:
                            nc.scalar.mul(tgt[:, col, :], g[:, col, :], w1)
                    if n != 0:
                        nc.vector.tensor_max(acc[:, :, :], acc[:, :, :], tgt[:, :, :])
                nc.sync.dma_start(outd[c, :, :, :], acc[:, :, :])
    nc.compile()
    return nc


def _get_program():
    global _compiled
    if _compiled is None:
        _compiled = _build_program()
    return _compiled


def kernel(x, transforms, mask):
    from concourse.bass_utils import run_bass_kernel_spmd

    x = np.asarray(x, dtype=np.float32)
    transforms = np.asarray(transforms, dtype=np.float32)
    mask = np.asarray(mask, dtype=np.float32)

    flat, wgt = _host_index_weights(transforms, mask)
    flat = flat.reshape(B, N, NPIX)
    wgt = wgt.reshape(B, N, NPIX)

    # pixel j of a core's half -> chunk c = j // REAL, slot pos = j % REAL;
    # gather lands at partition pos % 128, col pos // 128.
    j = np.arange(HALF)
    cix = j // REAL
    pos = j % REAL

    in_maps = []
    for core in range(8):
        b, half = divmod(core, 2)
        sl = slice(half * HALF, (half + 1) * HALF)
        fl = flat[b, :, sl]  # [N, HALF]
        wg = wgt[b, :, sl]

        idx16 = np.zeros((NCH, N, CHUNK), np.int16)
        wchunk = np.zeros((NCH, N, CHUNK), np.float32)
        idx16[cix, :, pos] = (fl.T - HALF).astype(np.int16)
        wchunk[cix, :, pos] = wg.T
        # sentinel slot (pos CHUNK-1) and padding keep idx 0 / weight 0.

        # wrap idx: list position q -> [16*g + q%16, q//16], replicated g=0..7
        q = np.arange(CHUNK)
        iw = np.zeros((NCH, N, 128, CHUNK // 16), np.int16)
        for g_ in range(8):
            iw[:, :, 16 * g_ + (q % 16), q // 16] = idx16
        # weight layout: slot q -> [q%128, q//128]
        wlay = np.zeros((NCH, N, 128, GCOLS), np.float32)
        wlay[:, :, q % 128, q // 128] = wchunk

        xflat = x[b].reshape(NPIX, C)
        in_maps.append(
            {
                "xt": xflat[:HALF],
                "xbo": xflat[HALF:],
                # device reads per chunk: [128, N*IDXF] / [128, N*GCOLS]
                "idxw": np.ascontiguousarray(iw.transpose(0, 2, 1, 3)).reshape(
                    NCH, 128, N * (CHUNK // 16)
                ),
                "wgtd": np.ascontiguousarray(wlay.transpose(0, 2, 1, 3)).reshape(
                    NCH, 128, N * GCOLS
                ),
            }
        )

    nc = _get_program()
    res = run_bass_kernel_spmd(nc, in_maps, list(range(8)))

    out = np.empty((B, H, W, C), np.float32)
    for core in range(8):
        b, half = divmod(core, 2)
        o = res.results[core]["outd"]  # [NCH, 128, GCOLS, C]
        # pixel j <- chunk cix[j], slot pos[j] at [pos%128, pos//128]
        pix = o[cix, pos % 128, pos // 128, :]  # [HALF, C]
        out[b, half * ROWS : (half + 1) * ROWS] = pix.reshape(ROWS, W, C)
    return out
